# revision 1
# baseline (speedup 1.0000x reference)
"""Trainium2 Bass kernel for single-head full-dim attention (nn_CasualSelfAttention).

Reference math (B=4, S=4096, D=768, fp32):
    q = x @ Wq.T + bq ; k = x @ Wk.T + bk ; v = x @ Wv.T + bv
    att = softmax(q @ k.T * D**-0.5)        # no mask
    y = att @ v
    y = y.transpose(0,2,1).reshape(B,S,D)   # element permutation
    out = y @ Wc.T + bc

Sharding (8 cores): core c = 2*b + h handles batch b with ALL 4096 queries but
only its half of the keys/values (rows h*2048:(h+1)*2048). Each core produces a
partial unnormalized yT [768, 4096] (features x queries) plus partial softmax
sums, with the value bias folded in linearly (bv x partial_sums). A pairwise
ReduceScatter(add) chunked by feature rows hands core h the fully-reduced
feature slice [384*h : 384*h+384] for all queries — exactly the rows of y.T
that the permutation maps to output rows [2048*h : 2048*h+2048]. After
normalizing by the (also-reduced) sums, the flat buffer IS y_perm row-major,
and the final projection runs locally. The RS is split into query-column
blocks (the last ones small) so communication overlaps attention compute and
the serial tail is short. Host pre-transposes/casts weights and activations to
bf16; matmuls are bf16 with fp32 accumulation, softmax in fp32 (logits are
bounded ~|1.8| so no max subtraction is needed).
"""

import numpy as np
import ml_dtypes

BF16 = ml_dtypes.bfloat16

B, S, D = 4, 4096, 768
SK = S // 2            # keys per core
P = 128
DT = D // P            # 6 feature tiles
KT = SK // P           # 16 key tiles
QC = 512               # query chunk width
NQC = S // QC          # 8 query chunks
BLOCKS = [(i, 1) for i in range(8)]   # RS blocks as (start_qc, n_qc)
FH = D // 2            # 384: feature rows per RS chunk
SCALE = float(D) ** -0.5
GROUPS = [[0, 1], [2, 3], [4, 5], [6, 7]]

_nc = None


def _build_program(phases="ABCDEF"):
    import concourse.bass as bass
    import concourse.mybir as mybir
    import concourse.tile as tile
    from concourse import bacc

    f32 = mybir.dt.float32
    bf16 = mybir.dt.bfloat16
    Exp = mybir.ActivationFunctionType.Exp
    mult = mybir.AluOpType.mult
    add = mybir.AluOpType.add

    qc2blk = {}
    for bi, (s0, n) in enumerate(BLOCKS):
        for j in range(n):
            qc2blk[s0 + j] = (bi, j)

    nc = bacc.Bacc(None, num_devices=8)

    xq = nc.declare_dram_parameter("xq", [S, D], bf16, isOutput=False)
    xkv = nc.declare_dram_parameter("xkv", [SK, D], bf16, isOutput=False)
    wqT = nc.declare_dram_parameter("wqT", [D, D], bf16, isOutput=False)
    wkT = nc.declare_dram_parameter("wkT", [D, D], bf16, isOutput=False)
    wvT = nc.declare_dram_parameter("wvT", [D, D], bf16, isOutput=False)
    wcT = nc.declare_dram_parameter("wcT", [D, D], bf16, isOutput=False)
    bq = nc.declare_dram_parameter("bq", [D, 1], f32, isOutput=False)
    bk = nc.declare_dram_parameter("bk", [D, 1], f32, isOutput=False)
    bv = nc.declare_dram_parameter("bv", [D, 1], f32, isOutput=False)
    bc = nc.declare_dram_parameter("bc", [1, D], f32, isOutput=False)
    out = nc.declare_dram_parameter("out", [SK, D], f32, isOutput=True)

    def wload(dst, src):
        # [768, 768] row-major -> [128, 6, 768] with logical row g*128+p
        nc.sync.dma_start(dst[:], src[:].rearrange("(g p) d -> p g d", p=P))

    with tile.TileContext(nc) as tc:
        with tc.tile_pool(name="persist", bufs=1) as pp, \
             tc.tile_pool(name="dram", bufs=1, space="DRAM") as dram:
            # Per column block: rows 0:384 = feats 0:384, row 384 = partial
            # sums, rows 385:769 = feats 384:768, row 769 = partial sums.
            yTaug = [dram.tile([2 * (FH + 1), n * QC], f32, name=f"yTaug{b}", tag=f"yTaug{b}")
                     for b, (_, n) in enumerate(BLOCKS)]
            rs_out = [dram.tile([FH + 1, n * QC], f32, name=f"rs_out{b}", tag=f"rs_out{b}")
                      for b, (_, n) in enumerate(BLOCKS)]
            f_dram = dram.tile([SK, D], bf16)

            # persistent SBUF: kT/qT/v activations + biases + output weights
            kT_sb = [pp.tile([P, SK], bf16, name=f"kT{g}", tag=f"kT{g}") for g in range(DT)]
            qT_sb = [pp.tile([P, S], bf16, name=f"qT{g}", tag=f"qT{g}") for g in range(DT)]
            v_sb = [pp.tile([P, D], bf16, name=f"v{t}", tag=f"v{t}") for t in range(KT)]
            bq_sb = [pp.tile([P, 1], f32, name=f"bq{g}", tag=f"bq{g}") for g in range(DT)]
            bk_sb = [pp.tile([P, 1], f32, name=f"bk{g}", tag=f"bk{g}") for g in range(DT)]
            bv_sb = [pp.tile([P, 1], f32, name=f"bv{g}", tag=f"bv{g}") for g in range(DT)]
            ones_sb = pp.tile([P, P], f32, name="ones", tag="ones")
            nc.vector.memset(ones_sb[:], 1.0)
            for g in range(DT):
                nc.sync.dma_start(bq_sb[g][:], bq[g * P:(g + 1) * P, :])
                nc.sync.dma_start(bk_sb[g][:], bk[g * P:(g + 1) * P, :])
                nc.sync.dma_start(bv_sb[g][:], bv[g * P:(g + 1) * P, :])
            wc_sb = pp.tile([P, DT, D], bf16, tag="wc_sb")
            wload(wc_sb, wcT)
            bc_sb = pp.tile([1, D], f32, tag="bc_sb")
            nc.sync.dma_start(bc_sb[:], bc[:])
            bcb = pp.tile([P, D], f32, tag="bcb")
            nc.gpsimd.partition_broadcast(bcb[:], bc_sb[:])

            # ---- Phase A: kT [768, 2048] and v [2048, 768] from xkv ----
            if "A" in phases:
                import contextlib
                _ab_stack = contextlib.ExitStack()
                pa = _ab_stack.enter_context(tc.tile_pool(name="pA", bufs=1))
                with tc.tile_pool(name="psA", bufs=2, space="PSUM") as psa:
                    wk_sb = pa.tile([P, DT, D], bf16, tag="wk_sb")
                    wload(wk_sb, wkT)
                    wv_sb = pa.tile([P, DT, D], bf16, tag="wv_sb")
                    wload(wv_sb, wvT)
                    for c in range(SK // QC):
                        xkvT = pa.tile([P, DT, QC], bf16, tag="xkvT", bufs=2, name="xkvT")
                        nc.sync.dma_start_transpose(xkvT[:], xkv[c * QC:(c + 1) * QC, :])
                        for go in range(DT):
                            ps = psa.tile([P, QC], f32, tag="pk")
                            for gi in range(DT):
                                nc.tensor.matmul(
                                    ps[:], wk_sb[:, gi, go * P:(go + 1) * P],
                                    xkvT[:, gi, :],
                                    start=(gi == 0), stop=(gi == DT - 1))
                            nc.vector.tensor_scalar_add(
                                kT_sb[go][:, c * QC:(c + 1) * QC], ps[:], bk_sb[go][:])
                        for tl in range(4):
                            t = c * 4 + tl
                            for half in range(2):
                                ps = psa.tile([P, FH], f32, tag="pv")
                                for gi in range(DT):
                                    nc.tensor.matmul(
                                        ps[:], xkvT[:, gi, tl * P:(tl + 1) * P],
                                        wv_sb[:, gi, half * FH:(half + 1) * FH],
                                        start=(gi == 0), stop=(gi == DT - 1))
                                nc.vector.tensor_copy(v_sb[t][:, half * FH:(half + 1) * FH], ps[:])

            # ---- Phase B: qT [768, 4096] from xq ----
            if "B" in phases:
                with tc.tile_pool(name="psB", bufs=2, space="PSUM") as psb:
                    pb = pa
                    wq_sb = pb.tile([P, DT, D], bf16, tag="wq_sb")
                    wload(wq_sb, wqT)
                    for c in range(NQC):
                        xqT = pb.tile([P, DT, QC], bf16, tag="xqT", bufs=3, name="xqT")
                        nc.sync.dma_start_transpose(xqT[:], xq[c * QC:(c + 1) * QC, :])
                        for go in range(DT):
                            ps = psb.tile([P, QC], f32, tag="pq")
                            for gi in range(DT):
                                nc.tensor.matmul(
                                    ps[:], wq_sb[:, gi, go * P:(go + 1) * P],
                                    xqT[:, gi, :],
                                    start=(gi == 0), stop=(gi == DT - 1))
                            nc.vector.tensor_scalar_add(
                                qT_sb[go][:, c * QC:(c + 1) * QC], ps[:], bq_sb[go][:])
                _ab_stack.close()

            # ---- Phase C: attention; write yTaug; chunked RS + normalize ----
            if "C" in phases:
                with tc.tile_pool(name="pC", bufs=2) as pc, \
                     tc.tile_pool(name="pE", bufs=2) as pe, \
                     tc.tile_pool(name="psC", bufs=1, space="PSUM") as psc:
                    f_view = f_dram[:].rearrange("a b -> (a b)").rearrange(
                        "(x c) -> x c", c=S)

                    MAXBW = max(n for _, n in BLOCKS) * QC

                    def emit_norm(b):
                        bw = BLOCKS[b][1] * QC
                        c0 = BLOCKS[b][0] * QC
                        s_row = pe.tile([1, MAXBW], f32, tag="s_row", name="s_row")
                        nc.sync.dma_start(s_row[:, :bw], rs_out[b][FH:FH + 1, :])
                        rec = pe.tile([1, MAXBW], f32, tag="rec", name="rec")
                        nc.vector.reciprocal(rec[:, :bw], s_row[:, :bw])
                        rbc = pe.tile([P, MAXBW], f32, tag="rbc", name="rbc")
                        nc.gpsimd.partition_broadcast(rbc[:, :bw], rec[:, :bw])
                        for r in range(FH // P):
                            fr = pe.tile([P, MAXBW], f32, tag="fr", bufs=2, name="fr")
                            nc.sync.dma_start(fr[:, :bw], rs_out[b][r * P:(r + 1) * P, :])
                            fn = pe.tile([P, MAXBW], bf16, tag="fn", bufs=2, name="fn")
                            nc.vector.tensor_mul(fn[:, :bw], fr[:, :bw], rbc[:, :bw])
                            nc.sync.dma_start(
                                f_view[r * P:(r + 1) * P, c0:c0 + bw], fn[:, :bw])

                    for qc in range(NQC):
                        blk, col = qc2blk[qc]
                        sums_acc = pc.tile([P, QC], f32, tag="sums_acc")
                        nc.vector.memset(sums_acc[:], 0.0)
                        ypsum = [psc.tile([P, QC], f32, name=f"y{e}", tag=f"y{e}", bufs=1)
                                 for e in range(DT)]
                        a_tiles = {}
                        for kt in range(KT):
                            aps = psc.tile([P, QC], f32, tag="att", bufs=2)
                            for gi in range(DT):
                                nc.tensor.matmul(
                                    aps[:], kT_sb[gi][:, kt * P:(kt + 1) * P],
                                    qT_sb[gi][:, qc * QC:(qc + 1) * QC],
                                    start=(gi == 0), stop=(gi == DT - 1))
                            # software pipeline: y-matmuls for kt-1 issue while
                            # the exp for kt is still on the scalar engine
                            if kt > 0:
                                for e in range(DT):
                                    nc.tensor.matmul(
                                        ypsum[e][:], v_sb[kt - 1][:, e * P:(e + 1) * P],
                                        a_tiles[kt - 1][:],
                                        start=(kt - 1 == 0), stop=False)
                            a_sb = pc.tile([P, QC], bf16, tag="a_sb", bufs=4)
                            a_tiles[kt] = a_sb
                            nc.scalar.activation(a_sb[:], aps[:], Exp, scale=SCALE)
                            nc.vector.tensor_add(sums_acc[:], sums_acc[:], a_sb[:])
                        for e in range(DT):
                            nc.tensor.matmul(
                                ypsum[e][:], v_sb[KT - 1][:, e * P:(e + 1) * P],
                                a_tiles[KT - 1][:],
                                start=False, stop=True)
                        # ones.T @ sums_acc both reduces across partitions and
                        # replicates the result onto all 128 partitions
                        sp = psc.tile([P, QC], f32, tag="att", bufs=2)
                        nc.tensor.matmul(sp[:], ones_sb[:], sums_acc[:], start=True, stop=True)
                        sbc = pc.tile([P, QC], f32, tag="sbc")
                        nc.vector.tensor_copy(sbc[:], sp[:])
                        yb = yTaug[blk]
                        nc.sync.dma_start(yb[FH:FH + 1, col * QC:(col + 1) * QC], sbc[0:1, :])
                        nc.sync.dma_start(yb[2 * FH + 1:2 * FH + 2, col * QC:(col + 1) * QC], sbc[0:1, :])
                        for e in range(DT):
                            yt_sb = pc.tile([P, QC], f32, tag="yt_sb", bufs=3)
                            # (sums_bcast * bv[e]) + ypsum  — folds the value bias
                            nc.vector.scalar_tensor_tensor(
                                yt_sb[:], sbc[:], bv_sb[e][:], ypsum[e][:], mult, add)
                            row = e * P if e < 3 else (FH + 1) + (e - 3) * P
                            nc.sync.dma_start(
                                yb[row:row + P, col * QC:(col + 1) * QC], yt_sb[:])

                        if col == BLOCKS[blk][1] - 1 and "D" in phases:
                            # block complete: reduce-scatter it. Normalization
                            # of block b-1 is emitted here (one block late) so
                            # nothing queues up behind an in-flight collective.
                            nc.gpsimd.collective_compute(
                                "ReduceScatter", mybir.AluOpType.add,
                                replica_groups=GROUPS,
                                ins=[yTaug[blk].opt()], outs=[rs_out[blk].opt()])
                            if "E" in phases and blk > 0:
                                emit_norm(blk - 1)

                    if "D" in phases and "E" in phases:
                        emit_norm(len(BLOCKS) - 1)

            # ---- Phase F: out = y_perm @ Wc.T + bc ----
            if "F" in phases:
                with tc.tile_pool(name="pF", bufs=1) as pf, \
                     tc.tile_pool(name="psF", bufs=2, space="PSUM") as psf:
                    fTs = []
                    for t in range(SK // P):
                        fT = pf.tile([P, DT, P], bf16, name=f"fT{t}", tag=f"fT{t}")
                        nc.sync.dma_start_transpose(fT[:], f_dram[t * P:(t + 1) * P, :])
                        fTs.append(fT)
                    for t in range(SK // P):
                        fT = fTs[t]
                        po = psf.tile([P, QC], f32, tag="po")
                        po2 = psf.tile([P, D - QC], f32, tag="po2")
                        for gi in range(DT):
                            nc.tensor.matmul(po[:], fT[:, gi, :], wc_sb[:, gi, 0:QC],
                                             start=(gi == 0), stop=(gi == DT - 1))
                            nc.tensor.matmul(po2[:], fT[:, gi, :], wc_sb[:, gi, QC:D],
                                             start=(gi == 0), stop=(gi == DT - 1))
                        o_sb = pf.tile([P, D], f32, tag="o_sb", bufs=3)
                        nc.vector.tensor_add(o_sb[:, 0:QC], po[:], bcb[:, 0:QC])
                        nc.vector.tensor_add(o_sb[:, QC:D], po2[:], bcb[:, QC:D])
                        nc.sync.dma_start(out[t * P:(t + 1) * P, :], o_sb[:])

    return nc


def _get_nc():
    global _nc
    if _nc is None:
        _nc = _build_program()
        _nc.finalize()
    return _nc


def _prep_in_maps(x, Wq, bq, Wk, bk, Wv, bv, Wc, bc):
    x = np.asarray(x, dtype=np.float32)
    wqT = np.ascontiguousarray(np.asarray(Wq, np.float32).T).astype(BF16)
    wkT = np.ascontiguousarray(np.asarray(Wk, np.float32).T).astype(BF16)
    wvT = np.ascontiguousarray(np.asarray(Wv, np.float32).T).astype(BF16)
    wcT = np.ascontiguousarray(np.asarray(Wc, np.float32).T).astype(BF16)
    bqc = np.asarray(bq, np.float32).reshape(D, 1).copy()
    bkc = np.asarray(bk, np.float32).reshape(D, 1).copy()
    bvc = np.asarray(bv, np.float32).reshape(D, 1).copy()
    bcc = np.asarray(bc, np.float32).reshape(1, D).copy()
    in_maps = []
    for c in range(8):
        b, h = divmod(c, 2)
        xb = x[b].astype(BF16)
        in_maps.append({
            "xq": xb,
            "xkv": np.ascontiguousarray(xb[h * SK:(h + 1) * SK]),
            "wqT": wqT, "wkT": wkT, "wvT": wvT, "wcT": wcT,
            "bq": bqc, "bk": bkc, "bv": bvc, "bc": bcc,
        })
    return in_maps


def _assemble(results):
    out = np.empty((B, S, D), dtype=np.float32)
    for c in range(8):
        b, h = divmod(c, 2)
        out[b, h * SK:(h + 1) * SK, :] = results[c]["out"]
    return out


def run_on_hw(trace=False, **inputs):
    from concourse.bass_utils import run_bass_kernel_spmd
    nc = _get_nc()
    in_maps = _prep_in_maps(**inputs)
    res = run_bass_kernel_spmd(nc, in_maps, list(range(8)), trace=trace)
    return _assemble(res.results), res


def kernel(**inputs):
    out, _ = run_on_hw(trace=False, **inputs)
    return out



# revision 2
# speedup vs baseline: 1.4065x; 1.4065x over previous
"""Trainium2 Bass kernel for single-head full-dim attention (nn_CasualSelfAttention).

Reference math (B=4, S=4096, D=768, fp32):
    q = x @ Wq.T + bq ; k = x @ Wk.T + bk ; v = x @ Wv.T + bv
    att = softmax(q @ k.T * D**-0.5)        # no mask
    y = att @ v
    y = y.transpose(0,2,1).reshape(B,S,D)   # element permutation
    out = y @ Wc.T + bc

Sharding (8 cores): core c = 2*b + h handles batch b with ALL 4096 queries but
only its half of the keys/values (rows h*2048:(h+1)*2048). Each core produces a
partial unnormalized yT [768, 4096] (features x queries) plus partial softmax
sums. A pairwise ReduceScatter(add), chunked by 512-query column blocks, hands
core h the fully-reduced feature slice [384*h : 384*h+384] for all queries —
exactly the rows of y.T that the permutation maps to output rows
[2048*h : 2048*h+2048]. After normalizing by the (also-reduced) sums and adding
bv (valid because sum(att)=den), the flat buffer IS y_perm row-major, and the
final projection runs locally.

v2 performance structure:
  - Host pre-transposes x (and weights) so all device loads are plain DMAs.
  - q/k projections and the q.k^T matmul run in fp8e4m3 with DoubleRow
    (2 k-tiles of 128 contracted per matmul, ~1.4x tensor throughput). The
    fp8 weights/biases are pre-scaled by 32 on the host to stay in e4m3's
    normal range; the exp() activation scale absorbs the 32*32 factor.
  - av matmul, v/c projections stay bf16 (fp8 there would put ~2% error on
    the output, over the tolerance).
  - ReduceScatter payloads are bf16 (half the wire time); a tiny dummy
    collective at kernel start absorbs the first-collective warmup cost.
  - Softmax sums ride in the yTaug tensor (rows 384/769) so one RS reduces
    both; bv is folded in post-normalization (num/den + bv).
  - Norm for block b is emitted a few instructions into qc b+2 so its DVE ops
    can never stall phase C's exp->sums->av chain.
  - Phase F loads y_perm^T via 4 large transposed DMAs overlapped with its
    matmuls.
"""

import numpy as np
import ml_dtypes

BF16 = ml_dtypes.bfloat16
F8 = ml_dtypes.float8_e4m3

B, S, D = 4, 4096, 768
SK = S // 2            # keys per core
P = 128
DT = D // P            # 6 feature tiles
KT = SK // P           # 16 key tiles
QC = 512               # query chunk width
NQC = S // QC          # 8 query chunks / RS blocks
FH = D // 2            # 384: feature rows per RS chunk
WS = 32.0              # host-side fp8 weight scale for Wq/Wk (and bq/bk)
SCALE = float(D) ** -0.5
EXP_SCALE = SCALE / (WS * WS)
GROUPS = [[0, 1], [2, 3], [4, 5], [6, 7]]

_nc = None


def _build_program():
    import concourse.bass as bass
    import concourse.mybir as mybir
    import concourse.tile as tile
    from concourse import bacc

    f32 = mybir.dt.float32
    bf16 = mybir.dt.bfloat16
    f8 = mybir.dt.float8e4
    Exp = mybir.ActivationFunctionType.Exp
    Copy = mybir.ActivationFunctionType.Copy
    DR = mybir.MatmulPerfMode.DoubleRow

    nc = bacc.Bacc(None, num_devices=8)

    xq8 = nc.declare_dram_parameter("xq8", [D, S], f8, isOutput=False)
    xkv8 = nc.declare_dram_parameter("xkv8", [D, SK], f8, isOutput=False)
    xkv16 = nc.declare_dram_parameter("xkv16", [D, SK], bf16, isOutput=False)
    wq8 = nc.declare_dram_parameter("wq8", [D, D], f8, isOutput=False)
    wk8 = nc.declare_dram_parameter("wk8", [D, D], f8, isOutput=False)
    wvT = nc.declare_dram_parameter("wvT", [D, D], bf16, isOutput=False)
    wcT = nc.declare_dram_parameter("wcT", [D, D], bf16, isOutput=False)
    bq = nc.declare_dram_parameter("bq", [D, 1], f32, isOutput=False)
    bk = nc.declare_dram_parameter("bk", [D, 1], f32, isOutput=False)
    bvh = nc.declare_dram_parameter("bvh", [FH, 1], f32, isOutput=False)
    bc = nc.declare_dram_parameter("bc", [1, D], f32, isOutput=False)
    out = nc.declare_dram_parameter("out", [SK, D], f32, isOutput=True)

    def wload(dst, src):
        # [768, 768] row-major -> [128, 6, 768] with logical row g*128+p
        nc.sync.dma_start(dst[:], src[:].rearrange("(g p) d -> p g d", p=P))

    def xload(dst, src, c0, w):
        # [768, S] cols c0:c0+w -> [128, 6, w]
        nc.sync.dma_start(dst[:], src[:, c0:c0 + w].rearrange("(g p) s -> p g s", p=P))

    with tile.TileContext(nc) as tc:
        with tc.tile_pool(name="persist", bufs=1) as pp, \
             tc.tile_pool(name="dram", bufs=1, space="DRAM") as dram:
            # Per column block: rows 0:384 = feats 0:384, row 384 = partial
            # sums, rows 385:769 = feats 384:768, row 769 = partial sums.
            yTaug = [dram.tile([2 * (FH + 1), QC], bf16, name=f"yTaug{b}", tag=f"yTaug{b}")
                     for b in range(NQC)]
            rs_out = [dram.tile([FH + 1, QC], bf16, name=f"rs_out{b}", tag=f"rs_out{b}")
                      for b in range(NQC)]
            f_dram = dram.tile([SK, D], bf16)
            warm_in = dram.tile([2, 64], bf16, name="warm_in", tag="warm_in")
            warm_out = dram.tile([1, 64], bf16, name="warm_out", tag="warm_out")

            # persistent SBUF
            kT_sb = pp.tile([P, DT, SK], f8, tag="kT")
            qT_sb = pp.tile([P, DT, S], f8, tag="qT")
            v_sb = [pp.tile([P, D], bf16, name=f"v{t}", tag=f"v{t}") for t in range(KT)]
            bq_sb = pp.tile([P, DT], f32, tag="bq_sb")
            bk_sb = pp.tile([P, DT], f32, tag="bk_sb")
            bvh_sb = pp.tile([P, 3], f32, tag="bvh_sb")
            ones_sb = pp.tile([P, P], bf16, name="ones", tag="ones")
            wc_sb = pp.tile([P, DT, D], bf16, tag="wc_sb")
            bc_sb = pp.tile([1, D], f32, tag="bc_sb")
            bcb = pp.tile([P, D], f32, tag="bcb")

            # ---- Phase A: kT [768x2048 fp8] and v [2048x768 bf16] ----
            import contextlib
            _ab_stack = contextlib.ExitStack()
            pa = _ab_stack.enter_context(tc.tile_pool(name="pA", bufs=1))
            with tc.tile_pool(name="psA", bufs=2, space="PSUM") as psa:
                wk_sb = pa.tile([P, DT, D], f8, tag="wk_sb")
                wload(wk_sb, wk8)
                nc.sync.dma_start(bk_sb[:], bk[:].rearrange("(g p) o -> p (g o)", p=P))
                # warm up the collectives stack while A computes
                ws_sb = pa.tile([2, 64], bf16, tag="ws_sb")
                nc.vector.memset(ws_sb[:], 0.0)
                nc.sync.dma_start(warm_in[:], ws_sb[:])
                nc.gpsimd.collective_compute(
                    "ReduceScatter", mybir.AluOpType.add,
                    replica_groups=GROUPS,
                    ins=[warm_in[:].opt()], outs=[warm_out[:].opt()])
                wv_sb = pa.tile([P, DT, D], bf16, tag="wv_sb")
                wload(wv_sb, wvT)
                nc.vector.memset(ones_sb[:], 1.0)
                nc.sync.dma_start(bvh_sb[:], bvh[:].rearrange("(r p) o -> p (r o)", p=P))
                for c in range(SK // QC):
                    x8c = pa.tile([P, DT, QC], f8, tag="x8c", bufs=2, name="x8c")
                    xload(x8c, xkv8, c * QC, QC)
                    for go in range(DT):
                        ps = psa.tile([P, QC], f32, tag="pk")
                        for a in range(3):
                            nc.tensor.matmul(
                                ps[:], wk_sb[:, 2 * a:2 * a + 2, go * P:(go + 1) * P],
                                x8c[:, 2 * a:2 * a + 2, :],
                                start=(a == 0), stop=(a == 2), perf_mode=DR)
                        nc.vector.tensor_scalar_add(
                            kT_sb[:, go, c * QC:(c + 1) * QC], ps[:],
                            bk_sb[:, go:go + 1])
                    x16c = pa.tile([P, DT, QC], bf16, tag="x16c", bufs=2, name="x16c")
                    xload(x16c, xkv16, c * QC, QC)
                    for tl in range(4):
                        t = c * 4 + tl
                        for half in range(2):
                            ps = psa.tile([P, FH], f32, tag="pv")
                            for gi in range(DT):
                                nc.tensor.matmul(
                                    ps[:], x16c[:, gi, tl * P:(tl + 1) * P],
                                    wv_sb[:, gi, half * FH:(half + 1) * FH],
                                    start=(gi == 0), stop=(gi == DT - 1))
                            nc.vector.tensor_copy(v_sb[t][:, half * FH:(half + 1) * FH], ps[:])

            # ---- Phase B: qT [768x4096 fp8] ----
            with tc.tile_pool(name="psB", bufs=3, space="PSUM") as psb:
                pb = pa
                wq_sb = pb.tile([P, DT, D], f8, tag="wq_sb")
                wload(wq_sb, wq8)
                nc.sync.dma_start(bq_sb[:], bq[:].rearrange("(g p) o -> p (g o)", p=P))
                wload(wc_sb, wcT)
                nc.sync.dma_start(bc_sb[:], bc[:])
                nc.gpsimd.partition_broadcast(bcb[:], bc_sb[:])
                for c in range(NQC):
                    x8c = pb.tile([P, DT, QC], f8, tag="xq8c", bufs=3, name="xq8c")
                    xload(x8c, xq8, c * QC, QC)
                    for go in range(DT):
                        ps = psb.tile([P, QC], f32, tag="pq")
                        for a in range(3):
                            nc.tensor.matmul(
                                ps[:], wq_sb[:, 2 * a:2 * a + 2, go * P:(go + 1) * P],
                                x8c[:, 2 * a:2 * a + 2, :],
                                start=(a == 0), stop=(a == 2), perf_mode=DR)
                        nc.vector.tensor_scalar_add(
                            qT_sb[:, go, c * QC:(c + 1) * QC], ps[:],
                            bq_sb[:, go:go + 1])
                _ab_stack.close()

            # ---- Phase C: attention; yTaug; chunked RS; late norm ----
            with tc.tile_pool(name="pC", bufs=2) as pc, \
                 tc.tile_pool(name="pE", bufs=2) as pe, \
                 tc.tile_pool(name="psC", bufs=1, space="PSUM") as psc:
                f_view = f_dram[:].rearrange("a b -> (a b)").rearrange(
                    "(x c) -> x c", c=S)

                def emit_norm(b):
                    c0 = b * QC
                    s_row = pe.tile([1, QC], bf16, tag="s_row", name="s_row")
                    nc.sync.dma_start(s_row[:], rs_out[b][FH:FH + 1, :])
                    den = pe.tile([P, QC], bf16, tag="den", name="den")
                    nc.gpsimd.partition_broadcast(den[:], s_row[:])
                    rec = pe.tile([P, QC], f32, tag="rec", name="rec")
                    nc.vector.reciprocal(rec[:], den[:])
                    for r in range(FH // P):
                        fr = pe.tile([P, QC], bf16, tag="fr", bufs=2, name="fr")
                        nc.sync.dma_start(fr[:], rs_out[b][r * P:(r + 1) * P, :])
                        tmp = pe.tile([P, QC], f32, tag="tmp", bufs=2, name="tmp")
                        nc.vector.tensor_mul(tmp[:], fr[:], rec[:])
                        fn = pe.tile([P, QC], bf16, tag="fn", bufs=2, name="fn")
                        nc.vector.tensor_scalar_add(fn[:], tmp[:], bvh_sb[:, r:r + 1])
                        nc.sync.dma_start(
                            f_view[r * P:(r + 1) * P, c0:c0 + QC], fn[:])

                pending = []
                for qc in range(NQC):
                    sums_acc = pc.tile([P, QC], bf16, tag="sums_acc")
                    nc.vector.memset(sums_acc[:], 0.0)
                    ypsum = [psc.tile([P, QC], f32, name=f"y{e}", tag=f"y{e}", bufs=1)
                             for e in range(DT)]
                    a_tiles = {}
                    for kt in range(KT):
                        if kt == 3 and pending:
                            emit_norm(pending.pop(0))
                        aps = psc.tile([P, QC], f32, tag="att", bufs=2)
                        for a in range(3):
                            nc.tensor.matmul(
                                aps[:], kT_sb[:, 2 * a:2 * a + 2, kt * P:(kt + 1) * P],
                                qT_sb[:, 2 * a:2 * a + 2, qc * QC:(qc + 1) * QC],
                                start=(a == 0), stop=(a == 2), perf_mode=DR)
                        # software pipeline: y-matmuls for kt-1 issue while
                        # the exp for kt is still on the scalar engine
                        if kt > 0:
                            for e in range(DT):
                                nc.tensor.matmul(
                                    ypsum[e][:], v_sb[kt - 1][:, e * P:(e + 1) * P],
                                    a_tiles[kt - 1][:],
                                    start=(kt - 1 == 0), stop=False)
                        a_sb = pc.tile([P, QC], bf16, tag="a_sb", bufs=6)
                        a_tiles[kt] = a_sb
                        nc.scalar.activation(a_sb[:], aps[:], Exp, scale=EXP_SCALE)
                        nc.vector.tensor_add(sums_acc[:], sums_acc[:], a_sb[:])
                    for e in range(DT):
                        nc.tensor.matmul(
                            ypsum[e][:], v_sb[KT - 1][:, e * P:(e + 1) * P],
                            a_tiles[KT - 1][:],
                            start=False, stop=True)
                    # ones.T @ sums_acc reduces across partitions and
                    # replicates the result onto all 128 partitions
                    sp = psc.tile([P, QC], f32, tag="att", bufs=2)
                    nc.tensor.matmul(sp[:], ones_sb[:], sums_acc[:], start=True, stop=True)
                    sbc = pc.tile([P, QC], bf16, tag="sbc")
                    nc.vector.tensor_copy(sbc[:], sp[:])
                    yb = yTaug[qc]
                    nc.sync.dma_start(yb[FH:FH + 1, :], sbc[0:1, :])
                    nc.sync.dma_start(yb[2 * FH + 1:2 * FH + 2, :], sbc[0:1, :])
                    for e in range(DT):
                        yt_sb = pc.tile([P, QC], bf16, tag="yt_sb", bufs=4)
                        # split the PSUM drain between Scalar and Vector so the
                        # next qc's av matmuls get their banks back quickly
                        if e < 3:
                            nc.scalar.activation(yt_sb[:], ypsum[e][:], Copy)
                        else:
                            nc.vector.tensor_copy(yt_sb[:], ypsum[e][:])
                        row = e * P if e < 3 else (FH + 1) + (e - 3) * P
                        nc.sync.dma_start(yb[row:row + P, :], yt_sb[:])

                    nc.gpsimd.collective_compute(
                        "ReduceScatter", mybir.AluOpType.add,
                        replica_groups=GROUPS,
                        ins=[yTaug[qc].opt()], outs=[rs_out[qc].opt()])
                    if qc > 0:
                        pending.append(qc - 1)

                for b in pending:
                    emit_norm(b)
                emit_norm(NQC - 1)

            # ---- Phase F: out = y_perm @ Wc.T + bc ----
            with tc.tile_pool(name="pF", bufs=1) as pf, \
                 tc.tile_pool(name="psF", bufs=2, space="PSUM") as psf:
                for tb in range(SK // QC):
                    fT = pf.tile([P, DT, QC], bf16, tag="fT", bufs=2, name="fT")
                    nc.sync.dma_start_transpose(fT[:], f_dram[tb * QC:(tb + 1) * QC, :])
                    for u in range(4):
                        t = tb * 4 + u
                        po = psf.tile([P, QC], f32, tag="po")
                        po2 = psf.tile([P, D - QC], f32, tag="po2")
                        for gi in range(DT):
                            nc.tensor.matmul(po[:], fT[:, gi, u * P:(u + 1) * P],
                                             wc_sb[:, gi, 0:QC],
                                             start=(gi == 0), stop=(gi == DT - 1))
                            nc.tensor.matmul(po2[:], fT[:, gi, u * P:(u + 1) * P],
                                             wc_sb[:, gi, QC:D],
                                             start=(gi == 0), stop=(gi == DT - 1))
                        o_sb = pf.tile([P, D], f32, tag="o_sb", bufs=3)
                        nc.vector.tensor_add(o_sb[:, 0:QC], po[:], bcb[:, 0:QC])
                        nc.vector.tensor_add(o_sb[:, QC:D], po2[:], bcb[:, QC:D])
                        nc.sync.dma_start(out[t * P:(t + 1) * P, :], o_sb[:])

    return nc


def _get_nc():
    global _nc
    if _nc is None:
        _nc = _build_program()
        _nc.finalize()
    return _nc


def _to_f8(a):
    return np.clip(a, -240.0, 240.0).astype(F8)


def _prep_in_maps(x, Wq, bq, Wk, bk, Wv, bv, Wc, bc):
    x = np.asarray(x, dtype=np.float32)
    wq8 = _to_f8(np.ascontiguousarray(np.asarray(Wq, np.float32).T) * WS)
    wk8 = _to_f8(np.ascontiguousarray(np.asarray(Wk, np.float32).T) * WS)
    wvT = np.ascontiguousarray(np.asarray(Wv, np.float32).T).astype(BF16)
    wcT = np.ascontiguousarray(np.asarray(Wc, np.float32).T).astype(BF16)
    bqc = (np.asarray(bq, np.float32) * WS).reshape(D, 1).copy()
    bkc = (np.asarray(bk, np.float32) * WS).reshape(D, 1).copy()
    bvc = np.asarray(bv, np.float32).reshape(D)
    bcc = np.asarray(bc, np.float32).reshape(1, D).copy()
    in_maps = []
    for c in range(8):
        b, h = divmod(c, 2)
        xT = np.ascontiguousarray(x[b].T)          # [D, S]
        xT8 = _to_f8(xT)
        kv8 = np.ascontiguousarray(xT8[:, h * SK:(h + 1) * SK])
        kv16 = np.ascontiguousarray(xT[:, h * SK:(h + 1) * SK]).astype(BF16)
        in_maps.append({
            "xq8": xT8, "xkv8": kv8, "xkv16": kv16,
            "wq8": wq8, "wk8": wk8, "wvT": wvT, "wcT": wcT,
            "bq": bqc, "bk": bkc,
            "bvh": np.ascontiguousarray(bvc[h * FH:(h + 1) * FH]).reshape(FH, 1),
            "bc": bcc,
        })
    return in_maps


def _assemble(results):
    out = np.empty((B, S, D), dtype=np.float32)
    for c in range(8):
        b, h = divmod(c, 2)
        out[b, h * SK:(h + 1) * SK, :] = results[c]["out"]
    return out


def run_on_hw(trace=False, **inputs):
    from concourse.bass_utils import run_bass_kernel_spmd
    nc = _get_nc()
    in_maps = _prep_in_maps(**inputs)
    res = run_bass_kernel_spmd(nc, in_maps, list(range(8)), trace=trace)
    return _assemble(res.results), res


def kernel(**inputs):
    out, _ = run_on_hw(trace=False, **inputs)
    return out


# revision 9
# speedup vs baseline: 1.4994x; 1.0660x over previous
"""Trainium2 Bass kernel for single-head full-dim attention (nn_CasualSelfAttention).

Reference math (B=4, S=4096, D=768, fp32):
    q = x @ Wq.T + bq ; k = x @ Wk.T + bk ; v = x @ Wv.T + bv
    att = softmax(q @ k.T * D**-0.5)        # no mask
    y = att @ v
    y = y.transpose(0,2,1).reshape(B,S,D)   # element permutation
    out = y @ Wc.T + bc

Sharding (8 cores): core c = 2*b + h handles batch b with ALL 4096 queries but
only its half of the keys/values (rows h*2048:(h+1)*2048). Each core produces a
partial unnormalized yT [768, 4096] (features x queries) plus partial softmax
sums. A pairwise ReduceScatter(add), chunked by 512-query column blocks, hands
core h the fully-reduced feature slice [384*h : 384*h+384] for all queries —
exactly the rows of y.T that the permutation maps to output rows
[2048*h : 2048*h+2048]. After normalizing by the (also-reduced) sums and adding
bv (valid because sum(att)=den), the flat buffer IS y_perm row-major, and the
final projection runs locally.

v2 performance structure:
  - Host pre-transposes x (and weights) so all device loads are plain DMAs.
  - q/k projections and the q.k^T matmul run in fp8e4m3 with DoubleRow
    (2 k-tiles of 128 contracted per matmul, ~1.4x tensor throughput). The
    fp8 weights/biases are pre-scaled by 32 on the host to stay in e4m3's
    normal range; the exp() activation scale absorbs the 32*32 factor.
  - av matmul, v/c projections stay bf16 (fp8 there would put ~2% error on
    the output, over the tolerance).
  - ReduceScatter payloads are bf16 (half the wire time); a tiny dummy
    collective at kernel start absorbs the first-collective warmup cost.
  - Softmax sums ride in the yTaug tensor (rows 384/769) so one RS reduces
    both; bv is folded in post-normalization (num/den + bv).
  - Norm for block b is emitted a few instructions into qc b+2 so its DVE ops
    can never stall phase C's exp->sums->av chain.
  - Phase F loads y_perm^T via 4 large transposed DMAs overlapped with its
    matmuls.
"""

import numpy as np
import ml_dtypes

BF16 = ml_dtypes.bfloat16
F8 = ml_dtypes.float8_e4m3

B, S, D = 4, 4096, 768
SK = S // 2            # keys per core
P = 128
DT = D // P            # 6 feature tiles
KT = SK // P           # 16 key tiles
QC = 512               # query chunk width
NQC = S // QC          # 8 query chunks / RS blocks
FH = D // 2            # 384: feature rows per RS chunk
WS = 32.0              # host-side fp8 weight scale for Wq/Wk (and bq/bk)
SCALE = float(D) ** -0.5
EXP_SCALE = SCALE / (WS * WS)
GROUPS = [[0, 1], [2, 3], [4, 5], [6, 7]]

_nc = None


def _build_program():
    import concourse.bass as bass
    import concourse.mybir as mybir
    import concourse.tile as tile
    from concourse import bacc

    f32 = mybir.dt.float32
    bf16 = mybir.dt.bfloat16
    f8 = mybir.dt.float8e4
    Exp = mybir.ActivationFunctionType.Exp
    Copy = mybir.ActivationFunctionType.Copy
    DR = mybir.MatmulPerfMode.DoubleRow

    nc = bacc.Bacc(None, num_devices=8)

    xq8 = nc.declare_dram_parameter("xq8", [D, S], f8, isOutput=False)
    xkv8 = nc.declare_dram_parameter("xkv8", [D, SK], f8, isOutput=False)
    xkv16 = nc.declare_dram_parameter("xkv16", [D, SK], bf16, isOutput=False)
    wq8 = nc.declare_dram_parameter("wq8", [D, D], f8, isOutput=False)
    wk8 = nc.declare_dram_parameter("wk8", [D, D], f8, isOutput=False)
    wvT = nc.declare_dram_parameter("wvT", [D, D], bf16, isOutput=False)
    wcT = nc.declare_dram_parameter("wcT", [D, D], bf16, isOutput=False)
    bq = nc.declare_dram_parameter("bq", [D, 1], f32, isOutput=False)
    bk = nc.declare_dram_parameter("bk", [D, 1], f32, isOutput=False)
    bvh = nc.declare_dram_parameter("bvh", [FH, 1], f32, isOutput=False)
    bc = nc.declare_dram_parameter("bc", [1, D], f32, isOutput=False)
    out = nc.declare_dram_parameter("out", [SK, D], f32, isOutput=True)

    def wload(dst, src):
        # [768, 768] row-major -> [128, 6, 768] with logical row g*128+p
        nc.sync.dma_start(dst[:], src[:].rearrange("(g p) d -> p g d", p=P))

    def xload(dst, src, c0, w):
        # [768, S] cols c0:c0+w -> [128, 6, w]
        nc.sync.dma_start(dst[:], src[:, c0:c0 + w].rearrange("(g p) s -> p g s", p=P))

    with tile.TileContext(nc) as tc:
        with tc.tile_pool(name="persist", bufs=1) as pp, \
             tc.tile_pool(name="dram", bufs=1, space="DRAM") as dram:
            # Per column block: rows 0:384 = feats 0:384, row 384 = partial
            # sums, rows 385:769 = feats 384:768, row 769 = partial sums.
            yTaug = [dram.tile([2 * (FH + 1), QC], bf16, name=f"yTaug{b}", tag=f"yTaug{b}")
                     for b in range(NQC)]
            rs_out = [dram.tile([FH + 1, QC], bf16, name=f"rs_out{b}", tag=f"rs_out{b}")
                      for b in range(NQC)]
            f_dram = dram.tile([SK, D], bf16)
            # realistically-sized dummy collective to absorb first-RS warmup
            # (contents uninitialized; output unused)
            warm_in = dram.tile([2 * (FH + 1), QC], bf16, name="warm_in", tag="warm_in")
            warm_out = dram.tile([FH + 1, QC], bf16, name="warm_out", tag="warm_out")

            # persistent SBUF. qT keeps each 512-query chunk's six feature
            # tiles contiguous: the att matmul's moving operand wants the
            # DoubleRow k-pair at a small (<=512B) stride.
            kT_sb = pp.tile([P, DT, SK], f8, tag="kT")
            qT_sb = pp.tile([P, NQC, DT, QC], f8, tag="qT")
            v_sb = [pp.tile([P, D], bf16, name=f"v{t}", tag=f"v{t}") for t in range(KT)]
            bq_sb = pp.tile([P, DT], f32, tag="bq_sb")
            bk_sb = pp.tile([P, DT], f32, tag="bk_sb")
            bvh_sb = pp.tile([P, 3], f32, tag="bvh_sb")
            ones_sb = pp.tile([P, P], bf16, name="ones", tag="ones")
            wc_sb = pp.tile([P, DT, D], bf16, tag="wc_sb")
            bc_sb = pp.tile([1, D], f32, tag="bc_sb")
            bcb = pp.tile([P, D], f32, tag="bcb")

            # ---- Phase A: kT [768x2048 fp8] and v [2048x768 bf16] ----
            import contextlib
            _ab_stack = contextlib.ExitStack()
            pa = _ab_stack.enter_context(tc.tile_pool(name="pA", bufs=1))
            with tc.tile_pool(name="psA", bufs=2, space="PSUM") as psa:
                wk_sb = pa.tile([P, DT, D], f8, tag="wk_sb")
                wload(wk_sb, wk8)
                x8cs = []
                for c in range(SK // QC):
                    x8c = pa.tile([P, DT, QC], f8, tag=f"x8c{c}", name=f"x8c{c}")
                    xload(x8c, xkv8, c * QC, QC)
                    x8cs.append(x8c)
                nc.sync.dma_start(bk_sb[:], bk[:].rearrange("(g p) o -> p (g o)", p=P))
                # all k-proj chunks back-to-back (pure fp8-DR, no mode mixing)
                for c in range(SK // QC):
                    for go in range(DT):
                        ps = psa.tile([P, QC], f32, tag="pk")
                        for a in range(3):
                            nc.tensor.matmul(
                                ps[:], wk_sb[:, 2 * a:2 * a + 2, go * P:(go + 1) * P],
                                x8cs[c][:, 2 * a:2 * a + 2, :],
                                start=(a == 0), stop=(a == 2), perf_mode=DR)
                        nc.vector.tensor_scalar_add(
                            kT_sb[:, go, c * QC:(c + 1) * QC], ps[:],
                            bk_sb[:, go:go + 1])
                # warm up the collectives stack while A computes
                nc.gpsimd.collective_compute(
                    "ReduceScatter", mybir.AluOpType.add,
                    replica_groups=GROUPS,
                    ins=[warm_in[:].opt()], outs=[warm_out[:].opt()])
                wv_sb = pa.tile([P, DT, D], bf16, tag="wv_sb")
                wload(wv_sb, wvT)
                nc.vector.memset(ones_sb[:], 1.0)
                nc.sync.dma_start(bvh_sb[:], bvh[:].rearrange("(r p) o -> p (r o)", p=P))
                for c in range(SK // QC):
                    x16c = pa.tile([P, DT, QC], bf16, tag="x16c", bufs=2, name="x16c")
                    xload(x16c, xkv16, c * QC, QC)
                    for tl in range(4):
                        t = c * 4 + tl
                        for half in range(2):
                            ps = psa.tile([P, FH], f32, tag="pv")
                            for gi in range(DT):
                                nc.tensor.matmul(
                                    ps[:], x16c[:, gi, tl * P:(tl + 1) * P],
                                    wv_sb[:, gi, half * FH:(half + 1) * FH],
                                    start=(gi == 0), stop=(gi == DT - 1))
                            nc.vector.tensor_copy(v_sb[t][:, half * FH:(half + 1) * FH], ps[:])

            # ---- Phase B: qT [768x4096 fp8] ----
            with tc.tile_pool(name="psB", bufs=3, space="PSUM") as psb:
                pb = pa
                wq_sb = pb.tile([P, DT, D], f8, tag="wq_sb")
                wload(wq_sb, wq8)
                nc.sync.dma_start(bq_sb[:], bq[:].rearrange("(g p) o -> p (g o)", p=P))
                wload(wc_sb, wcT)
                nc.sync.dma_start(bc_sb[:], bc[:])
                nc.gpsimd.partition_broadcast(bcb[:], bc_sb[:])
                for c in range(NQC):
                    x8c = pb.tile([P, DT, QC], f8, tag="xq8c", bufs=3, name="xq8c")
                    xload(x8c, xq8, c * QC, QC)
                    for go in range(DT):
                        ps = psb.tile([P, QC], f32, tag="pq")
                        for a in range(3):
                            nc.tensor.matmul(
                                ps[:], wq_sb[:, 2 * a:2 * a + 2, go * P:(go + 1) * P],
                                x8c[:, 2 * a:2 * a + 2, :],
                                start=(a == 0), stop=(a == 2), perf_mode=DR)
                        nc.vector.tensor_scalar_add(
                            qT_sb[:, c, go, :], ps[:],
                            bq_sb[:, go:go + 1])
                _ab_stack.close()

            # ---- Phase C: attention; yTaug; chunked RS; late norm ----
            with tc.tile_pool(name="pC", bufs=2) as pc, \
                 tc.tile_pool(name="pE", bufs=2) as pe, \
                 tc.tile_pool(name="psC", bufs=1, space="PSUM") as psc:
                f_view = f_dram[:].rearrange("a b -> (a b)").rearrange(
                    "(x c) -> x c", c=S)

                def emit_norm(b):
                    c0 = b * QC
                    s_row = pe.tile([1, QC], bf16, tag="s_row", name="s_row")
                    nc.sync.dma_start(s_row[:], rs_out[b][FH:FH + 1, :])
                    den = pe.tile([P, QC], bf16, tag="den", name="den")
                    nc.gpsimd.partition_broadcast(den[:], s_row[:])
                    den32 = pe.tile([P, QC], f32, tag="den32", name="den32")
                    nc.vector.tensor_copy(den32[:], den[:])
                    rec = pe.tile([P, QC], f32, tag="rec", name="rec")
                    nc.vector.reciprocal_approx_fast(rec[:], den32[:])
                    for r in range(FH // P):
                        fr = pe.tile([P, QC], bf16, tag="fr", bufs=2, name="fr")
                        nc.sync.dma_start(fr[:], rs_out[b][r * P:(r + 1) * P, :])
                        tmp = pe.tile([P, QC], f32, tag="tmp", bufs=2, name="tmp")
                        nc.vector.tensor_mul(tmp[:], fr[:], rec[:])
                        fn = pe.tile([P, QC], bf16, tag="fn", bufs=2, name="fn")
                        nc.vector.tensor_scalar_add(fn[:], tmp[:], bvh_sb[:, r:r + 1])
                        nc.sync.dma_start(
                            f_view[r * P:(r + 1) * P, c0:c0 + QC], fn[:])

                pending = []
                for qc in range(NQC):
                    sums_acc = pc.tile([P, QC], bf16, tag="sums_acc")
                    nc.vector.memset(sums_acc[:], 0.0)
                    ypsum = [psc.tile([P, QC], f32, name=f"y{e}", tag=f"y{e}", bufs=1)
                             for e in range(DT)]
                    a_tiles = {}
                    for kt in range(KT):
                        if kt == 3 and pending:
                            emit_norm(pending.pop(0))
                        aps = psc.tile([P, QC], f32, tag="att", bufs=2)
                        for a in range(3):
                            nc.tensor.matmul(
                                aps[:], kT_sb[:, 2 * a:2 * a + 2, kt * P:(kt + 1) * P],
                                qT_sb[:, qc, 2 * a:2 * a + 2, :],
                                start=(a == 0), stop=(a == 2), perf_mode=DR)
                        # software pipeline: y-matmuls for kt-1 issue while
                        # the exp for kt is still on the scalar engine
                        if kt > 0:
                            for e in range(DT):
                                nc.tensor.matmul(
                                    ypsum[e][:], v_sb[kt - 1][:, e * P:(e + 1) * P],
                                    a_tiles[kt - 1][:],
                                    start=(kt - 1 == 0), stop=False)
                        a_sb = pc.tile([P, QC], bf16, tag="a_sb", bufs=6)
                        a_tiles[kt] = a_sb
                        nc.scalar.activation(a_sb[:], aps[:], Exp, scale=EXP_SCALE)
                        nc.vector.tensor_add(sums_acc[:], sums_acc[:], a_sb[:])
                    for e in range(DT):
                        nc.tensor.matmul(
                            ypsum[e][:], v_sb[KT - 1][:, e * P:(e + 1) * P],
                            a_tiles[KT - 1][:],
                            start=False, stop=True)
                    # ones.T @ sums_acc reduces across partitions and
                    # replicates the result onto all 128 partitions
                    sp = psc.tile([P, QC], f32, tag="att", bufs=2)
                    nc.tensor.matmul(sp[:], ones_sb[:], sums_acc[:], start=True, stop=True)
                    yb = yTaug[qc]
                    # drain ypsum banks in consumption order, alternating
                    # Scalar/Vector so the next qc's av matmuls free up fast
                    for e in range(DT):
                        yt_sb = pc.tile([P, QC], bf16, tag="yt_sb", bufs=4)
                        if e % 2 == 0:
                            nc.scalar.activation(yt_sb[:], ypsum[e][:], Copy)
                        else:
                            nc.vector.tensor_copy(yt_sb[:], ypsum[e][:])
                        row = e * P if e < 3 else (FH + 1) + (e - 3) * P
                        nc.sync.dma_start(yb[row:row + P, :], yt_sb[:])
                    sbc = pc.tile([P, QC], bf16, tag="sbc")
                    nc.vector.tensor_copy(sbc[:], sp[:])
                    nc.sync.dma_start(yb[FH:FH + 1, :], sbc[0:1, :])
                    nc.sync.dma_start(yb[2 * FH + 1:2 * FH + 2, :], sbc[0:1, :])

                    nc.gpsimd.collective_compute(
                        "ReduceScatter", mybir.AluOpType.add,
                        replica_groups=GROUPS,
                        ins=[yTaug[qc].opt()], outs=[rs_out[qc].opt()])
                    if qc > 0:
                        pending.append(qc - 1)

                for b in pending:
                    emit_norm(b)
                emit_norm(NQC - 1)

            # ---- Phase F: out = y_perm @ Wc.T + bc ----
            with tc.tile_pool(name="pF", bufs=1) as pf, \
                 tc.tile_pool(name="psF", bufs=2, space="PSUM") as psf:
                for tb in range(SK // QC):
                    fT = pf.tile([P, DT, QC], bf16, tag="fT", bufs=2, name="fT")
                    nc.sync.dma_start_transpose(fT[:], f_dram[tb * QC:(tb + 1) * QC, :])
                    for u in range(4):
                        t = tb * 4 + u
                        po = psf.tile([P, QC], f32, tag="po")
                        po2 = psf.tile([P, D - QC], f32, tag="po2")
                        for gi in range(DT):
                            nc.tensor.matmul(po[:], fT[:, gi, u * P:(u + 1) * P],
                                             wc_sb[:, gi, 0:QC],
                                             start=(gi == 0), stop=(gi == DT - 1))
                            nc.tensor.matmul(po2[:], fT[:, gi, u * P:(u + 1) * P],
                                             wc_sb[:, gi, QC:D],
                                             start=(gi == 0), stop=(gi == DT - 1))
                        o_sb = pf.tile([P, D], f32, tag="o_sb", bufs=3)
                        nc.vector.tensor_add(o_sb[:, 0:QC], po[:], bcb[:, 0:QC])
                        nc.vector.tensor_add(o_sb[:, QC:D], po2[:], bcb[:, QC:D])
                        # out-writes go on the scalar HWDGE queue so the sync
                        # queue stays free for the fT transposes
                        nc.scalar.dma_start(out[t * P:(t + 1) * P, :], o_sb[:])

    return nc


def _get_nc():
    global _nc
    if _nc is None:
        _nc = _build_program()
        _nc.finalize()
    return _nc


def _to_f8(a):
    return np.clip(a, -240.0, 240.0).astype(F8)


def _prep_in_maps(x, Wq, bq, Wk, bk, Wv, bv, Wc, bc):
    x = np.asarray(x, dtype=np.float32)
    wq8 = _to_f8(np.ascontiguousarray(np.asarray(Wq, np.float32).T) * WS)
    wk8 = _to_f8(np.ascontiguousarray(np.asarray(Wk, np.float32).T) * WS)
    wvT = np.ascontiguousarray(np.asarray(Wv, np.float32).T).astype(BF16)
    wcT = np.ascontiguousarray(np.asarray(Wc, np.float32).T).astype(BF16)
    bqc = (np.asarray(bq, np.float32) * WS).reshape(D, 1).copy()
    bkc = (np.asarray(bk, np.float32) * WS).reshape(D, 1).copy()
    bvc = np.asarray(bv, np.float32).reshape(D)
    bcc = np.asarray(bc, np.float32).reshape(1, D).copy()
    in_maps = []
    for c in range(8):
        b, h = divmod(c, 2)
        xT = np.ascontiguousarray(x[b].T)          # [D, S]
        xT8 = _to_f8(xT)
        kv8 = np.ascontiguousarray(xT8[:, h * SK:(h + 1) * SK])
        kv16 = np.ascontiguousarray(xT[:, h * SK:(h + 1) * SK]).astype(BF16)
        in_maps.append({
            "xq8": xT8, "xkv8": kv8, "xkv16": kv16,
            "wq8": wq8, "wk8": wk8, "wvT": wvT, "wcT": wcT,
            "bq": bqc, "bk": bkc,
            "bvh": np.ascontiguousarray(bvc[h * FH:(h + 1) * FH]).reshape(FH, 1),
            "bc": bcc,
        })
    return in_maps


def _assemble(results):
    out = np.empty((B, S, D), dtype=np.float32)
    for c in range(8):
        b, h = divmod(c, 2)
        out[b, h * SK:(h + 1) * SK, :] = results[c]["out"]
    return out


def run_on_hw(trace=False, **inputs):
    from concourse.bass_utils import run_bass_kernel_spmd
    nc = _get_nc()
    in_maps = _prep_in_maps(**inputs)
    res = run_bass_kernel_spmd(nc, in_maps, list(range(8)), trace=trace)
    return _assemble(res.results), res


def kernel(**inputs):
    out, _ = run_on_hw(trace=False, **inputs)
    return out
